# revision 24
# baseline (speedup 1.0000x reference)
"""Trainium2 Bass kernel for nn_H_DYNA_42348377538865 (scatter_memory GRU + memory attention).

Self-contained: shards node dim N=512 across 8 NeuronCores (64 nodes/core),
runs a fully-unrolled 24-step recurrence per core, gathers on host.

Layout: feature-on-partitions, (node, batch) on free dim (col = n_local*32 + b,
NB=2048 cols/core). Key decompositions (validated vs reference in numpy):
  - rolling q-cache: q(h_t) computed once/step; 12 slots in 3x[128,2048] tiles;
    slot j pairs with memory slice s=(j-t)%12 -> 12 precomputed rotation stacks
  - softmax without max-subtraction; ONE fused matmul [96,128] emits means
    (rows 0:64) and replicated sums (rows 64:128) so the reciprocal broadcasts
  - single act table: sigmoid(x) = 0.5*(1+tanh(x/2)); exp/tanh/copy all live
    in the `exp_and_others` table -> zero table reloads
  - z/r gates in ONE [65,128] matmul + ONE tanh; GRU update via
    scalar_tensor_tensor: 2rh=(1+v)h (0.5 folded into Wc), w=hc-h, s=(1+u)w,
    h+=0.5s
  - q bias bq folded into the exp bias (sum_s bq.mem, constant per mem row)
  - decode: Wo folded into gate weights (no autoregressive y->x loopback);
    y accumulates in a persistent PSUM bank, copied out once at the end
  - hypernet nsw = node_emb @ weight_pool precomputed on host (param repack)
"""
import numpy as np
import sys

for _p in ("/opt/trn_rl_repo",):
    if _p not in sys.path:
        sys.path.append(_p)

import concourse.bass as bass
import concourse.bacc as bacc
import concourse.mybir as mybir
import concourse.tile as tile
from concourse import bass_utils

B, T, HORIZON, N = 32, 12, 12, 512
IN, OUT, H, P = 1, 1, 64, 32
S, ML, MG, DE = 12, 64, 32, 10
NCORES = 8
NL = N // NCORES        # 64
NB = NL * B             # 2048
NSTEP = T + HORIZON     # 24
CH = 4                  # column chunks
CW = NB // CH           # 512

F32 = mybir.dt.float32
BF16 = mybir.dt.bfloat16
AF = mybir.ActivationFunctionType
ALU = mybir.AluOpType


def build_nc():
    nc = bacc.Bacc("TRN2", target_bir_lowering=False, debug=False)
    d = {}
    d["xsrc"] = nc.dram_tensor("xsrc", [T, NB], BF16, kind="ExternalInput")
    d["memstack"] = nc.dram_tensor("memstack", [128, S * 3 * 96], BF16, kind="ExternalInput")
    d["nsw"] = nc.dram_tensor("nsw", [64, NL * 64], BF16, kind="ExternalInput")
    d["fms"] = nc.dram_tensor("fms", [96, 128], BF16, kind="ExternalInput")
    d["zrw"] = nc.dram_tensor("zrw", [65, 128], BF16, kind="ExternalInput")
    d["zrwf"] = nc.dram_tensor("zrwf", [64, 128], BF16, kind="ExternalInput")
    d["cws"] = nc.dram_tensor("cws", [65, 64], BF16, kind="ExternalInput")
    d["cwf"] = nc.dram_tensor("cwf", [64, 64], BF16, kind="ExternalInput")
    d["cwx"] = nc.dram_tensor("cwx", [64, 64], BF16, kind="ExternalInput")
    d["qw"] = nc.dram_tensor("qw", [64, 32], BF16, kind="ExternalInput")
    d["owd"] = nc.dram_tensor("owd", [64, HORIZON * HORIZON], BF16, kind="ExternalInput")
    d["bqlog"] = nc.dram_tensor("bqlog", [96, 1], F32, kind="ExternalInput")
    d["bzr2"] = nc.dram_tensor("bzr2", [128, 1], F32, kind="ExternalInput")
    d["bzrf2"] = nc.dram_tensor("bzrf2", [128, 1], F32, kind="ExternalInput")
    d["bce"] = nc.dram_tensor("bce", [64, 1], F32, kind="ExternalInput")
    d["bcd"] = nc.dram_tensor("bcd", [64, 1], F32, kind="ExternalInput")
    ys_d = nc.dram_tensor("ys", [HORIZON, NB], BF16, kind="ExternalOutput")

    with tile.TileContext(nc) as tc:
        with (
            tc.tile_pool(name="consts", bufs=1) as cp,
            tc.tile_pool(name="sp", bufs=4) as sp,
            tc.tile_pool(name="ps", bufs=2, space="PSUM") as pp,
            tc.tile_pool(name="pq", bufs=1, space="PSUM") as pq,
            tc.tile_pool(name="py", bufs=1, space="PSUM") as py,
        ):
            xs = cp.tile([T, NB], BF16)
            nc.sync.dma_start(xs[:], d["xsrc"].ap())
            msk = cp.tile([128, S * 3 * 96], BF16)
            nc.sync.dma_start(msk[:], d["memstack"].ap())
            nsw = cp.tile([64, NL * 64], BF16)
            nc.sync.dma_start(nsw[:], d["nsw"].ap())
            fms = cp.tile([96, 128], BF16)
            nc.sync.dma_start(fms[:], d["fms"].ap())
            zrw = cp.tile([65, 128], BF16)
            nc.sync.dma_start(zrw[:], d["zrw"].ap())
            zrwf = cp.tile([64, 128], BF16)
            nc.sync.dma_start(zrwf[:], d["zrwf"].ap())
            cws = cp.tile([65, 64], BF16)
            nc.sync.dma_start(cws[:], d["cws"].ap())
            cwf = cp.tile([64, 64], BF16)
            nc.sync.dma_start(cwf[:], d["cwf"].ap())
            cwx = cp.tile([64, 64], BF16)
            nc.sync.dma_start(cwx[:], d["cwx"].ap())
            qw = cp.tile([64, 32], BF16)
            nc.sync.dma_start(qw[:], d["qw"].ap())
            owd = cp.tile([64, HORIZON * HORIZON], BF16)
            nc.sync.dma_start(owd[:], d["owd"].ap())
            bqlog = cp.tile([96, 1], F32)
            nc.sync.dma_start(bqlog[:], d["bqlog"].ap())
            bzr2 = cp.tile([128, 1], F32)
            nc.sync.dma_start(bzr2[:], d["bzr2"].ap())
            bzrf2 = cp.tile([128, 1], F32)
            nc.sync.dma_start(bzrf2[:], d["bzrf2"].ap())
            bce = cp.tile([64, 1], F32)
            nc.sync.dma_start(bce[:], d["bce"].ap())
            bcd = cp.tile([64, 1], F32)
            nc.sync.dma_start(bcd[:], d["bcd"].ap())

            qb = []
            for g in range(3):
                q = cp.tile([128, NB], BF16, name=f"qb{g}")
                nc.vector.memset(q[:], 0.0)
                qb.append(q)
            hx = cp.tile([65, NB], BF16)
            nc.vector.memset(hx[:], 0.0)
            rhx = cp.tile([65, NB], BF16)
            nc.vector.memset(rhx[:], 0.0)
            # y staging: row 32c + d holds decode step d of chunk c
            # (32-aligned chunk bases for the ACT copy)
            ysb = cp.tile([128, CW], BF16)
            nc.sync.dma_start(hx[64:65, :], xs[0:1, :])
            nc.sync.dma_start(rhx[64:65, :], xs[0:1, :])

            # persistent PSUM: q projections (4 chunks x 32 rows), and the
            # decode y accumulator (chunk c rows 32c:32c+12; each decode step
            # adds Wo^T h into row 32c+d and +0 elsewhere)
            qpb = pq.tile([128, CW], F32)
            ypt = py.tile([128, CW], F32)

            csl = [slice(c * CW, (c + 1) * CW) for c in range(CH)]

            for t in range(NSTEP):
                r = t % S
                j = t % S
                g_w, row_w = j // 4, (j % 4) * 32
                enc = t <= T  # t==12 still uses x-row (x = source[:, -1])

                # --- PE: z|r gate logits ---
                zrp = []
                for c in range(CH):
                    zp = pp.tile([128, CW], F32, tag="fz")
                    if enc:
                        nc.tensor.matmul(zp[:], zrw[:], hx[:, csl[c]],
                                         start=True, stop=True)
                    else:
                        nc.tensor.matmul(zp[:], zrwf[:], hx[0:64, csl[c]],
                                         start=True, stop=True)
                    zrp.append(zp)
                # --- PE: attention logits from q-cache ---
                lgp = []
                for c in range(CH):
                    lg = pp.tile([96, CW], F32, tag="lg")
                    for g in range(3):
                        off = (r * 3 + g) * 96
                        nc.tensor.matmul(
                            lg[:], msk[:, off : off + 96], qb[g][:, csl[c]],
                            start=(g == 0), stop=(g == 2),
                        )
                    lgp.append(lg)
                # --- ACT: gates u|v = tanh((logits+b)/2) ---
                uvl = []
                for c in range(CH):
                    uv = sp.tile([128, CW], BF16, tag="uv", bufs=6)
                    nc.scalar.activation(uv[:], zrp[c][:], AF.Tanh,
                                         bias=(bzr2 if enc else bzrf2)[:, 0:1],
                                         scale=0.5)
                    uvl.append(uv)
                # --- ACT: exp of attention logits ---
                exl = []
                for c in range(CH):
                    ex = sp.tile([96, CW], BF16, tag="ex", bufs=6)
                    nc.scalar.activation(ex[:], lgp[c][:], AF.Exp,
                                         bias=bqlog[:, 0:1])
                    exl.append(ex)
                # --- DVE: 2*r*h = (1+v)*h  (v = r-gate tanh, rows 0:64) ---
                for c in range(CH):
                    nc.vector.scalar_tensor_tensor(
                        rhx[0:64, csl[c]], uvl[c][0:64, :], 1.0,
                        hx[0:64, csl[c]], ALU.add, ALU.mult)
                # --- PE: fused mean/sum matmul ---
                fzl = []
                for c in range(CH):
                    fz = pp.tile([128, CW], F32, tag="fz")
                    nc.tensor.matmul(fz[:], fms[:], exl[c][:],
                                     start=True, stop=True)
                    fzl.append(fz)
                # --- DVE: reciprocal of sums; fn = means * recip ---
                rtl = []
                for c in range(CH):
                    rt = sp.tile([64, CW], F32, tag="rt", bufs=5)
                    nc.vector.reciprocal_approx_fast(rt[:], fzl[c][64:128, :])
                    rtl.append(rt)
                fnl = []
                for c in range(CH):
                    fn = sp.tile([64, CW], BF16, tag="fn", bufs=5)
                    nc.vector.tensor_mul(fn[:], fzl[c][0:64, :], rtl[c][:])
                    fnl.append(fn)
                # --- PE: candidate pre-activation: Wc part then hypernet ---
                accl = []
                for c in range(CH):
                    acc = pp.tile([64, CW], F32, tag="acc")
                    if enc:
                        nc.tensor.matmul(acc[:], cws[:], rhx[:, csl[c]],
                                         start=True, stop=False,
                                         skip_group_check=True)
                    else:
                        nc.tensor.matmul(acc[:], cwf[:], rhx[0:64, csl[c]],
                                         start=True, stop=False,
                                         skip_group_check=True)
                        nc.tensor.matmul(acc[:], cwx[:], hx[0:64, csl[c]],
                                         start=False, stop=False,
                                         skip_group_check=True)
                    accl.append(acc)
                for c in range(CH):
                    for k in range(16):
                        n = c * 16 + k
                        nc.tensor.matmul(
                            accl[c][:, k * 32 : (k + 1) * 32],
                            nsw[:, n * 64 : (n + 1) * 64],
                            fnl[c][:, k * 32 : (k + 1) * 32],
                            start=False, stop=(k == 15), skip_group_check=True,
                        )
                # --- ACT: hc = tanh(acc + bc) ---
                hcl = []
                for c in range(CH):
                    hc = sp.tile([64, CW], BF16, tag="hc", bufs=5)
                    nc.scalar.activation(hc[:], accl[c][:], AF.Tanh,
                                         bias=(bce if enc else bcd)[:, 0:1])
                    hcl.append(hc)
                # --- DVE: h += 0.5*(1+u)*(hc-h)  (u = z-gate tanh, rows 64:128;
                # w lives at base partition 64 so the s-op inputs share a base) ---
                wl = []
                for c in range(CH):
                    w = sp.tile([128, CW], BF16, tag="w", bufs=5)
                    nc.gpsimd.tensor_sub(w[64:128, :], hcl[c][:], hx[0:64, csl[c]])
                    wl.append(w)
                sl = []
                for c in range(CH):
                    s2 = sp.tile([64, CW], BF16, tag="s2", bufs=5)
                    nc.vector.scalar_tensor_tensor(
                        s2[:], uvl[c][64:128, :], 1.0, wl[c][64:128, :],
                        ALU.add, ALU.mult)
                    sl.append(s2)
                for c in range(CH):
                    nc.vector.scalar_tensor_tensor(
                        hx[0:64, csl[c]], sl[c][:], 0.5, hx[0:64, csl[c]],
                        ALU.mult, ALU.add)
                # --- PE: q projection of new h; decode y projection ---
                if t < NSTEP - 1:
                    for c in range(CH):
                        nc.tensor.matmul(
                            qpb[32 * c : 32 * (c + 1), :], qw[:],
                            hx[0:64, csl[c]], start=True, stop=True,
                            tile_position=(0, 32 * c),
                        )
                if t >= T:
                    dstep = t - T
                    for c in range(CH):
                        nc.tensor.matmul(
                            ypt[32 * c : 32 * c + HORIZON, :],
                            owd[:, HORIZON * dstep : HORIZON * (dstep + 1)],
                            hx[0:64, csl[c]],
                            start=(dstep == 0), stop=(dstep == HORIZON - 1),
                            skip_group_check=True,
                            tile_position=(0, 32 * c),
                        )
                # --- DVE/ACT: q-cache slot update (gpsimd cannot read PSUM) ---
                if t < NSTEP - 1:
                    for c in range(CH):
                        dst = qb[g_w][row_w : row_w + 32, csl[c]]
                        src = qpb[32 * c : 32 * (c + 1), :]
                        if c < 2:
                            nc.vector.tensor_copy(dst, src)
                        else:
                            nc.scalar.activation(dst, src, AF.Copy)
                # --- DMA: encode x prefetch ---
                if t < T - 1:
                    nc.sync.dma_start(hx[64:65, :], xs[t + 1 : t + 2, :])
                    nc.sync.dma_start(rhx[64:65, :], xs[t + 1 : t + 2, :])

            for c in range(CH):
                nc.scalar.activation(
                    ysb[32 * c : 32 * c + HORIZON, :],
                    ypt[32 * c : 32 * c + HORIZON, :], AF.Copy)
            for c in range(CH):
                nc.sync.dma_start(
                    ys_d.ap()[0:HORIZON, c * CW : (c + 1) * CW],
                    ysb[32 * c : 32 * c + HORIZON, :])
    nc.compile()
    return nc


def precompute(inp):
    lm = np.asarray(inp["local_mem"], np.float64)
    gm = np.asarray(inp["global_mem"], np.float64)
    Wq = np.asarray(inp["Wq"], np.float64)
    bq = np.asarray(inp["bq"], np.float64)
    node_emb = np.asarray(inp["node_emb"], np.float64)
    wp = np.asarray(inp["weight_pool"], np.float64)
    Wz = np.asarray(inp["Wz"], np.float64)
    bz = np.asarray(inp["bz"], np.float64)
    Wr = np.asarray(inp["Wr"], np.float64)
    br = np.asarray(inp["br"], np.float64)
    Wc = np.asarray(inp["Wc"], np.float64)
    bc = np.asarray(inp["bc"], np.float64)
    Wo = np.asarray(inp["Wo"], np.float64)
    bo = np.asarray(inp["bo"], np.float64)

    c = {}
    c["nsw_full"] = np.einsum("nd,dfh->nfh", node_emb, wp)
    memsl = np.concatenate([lm.transpose(2, 0, 1), gm.transpose(2, 0, 1)], axis=1)  # [P,96,S]
    ms = np.zeros((128, S, 3, 96))
    for rr in range(S):
        for g in range(3):
            for i in range(4):
                s = (4 * g + i - rr) % S
                ms[32 * i : 32 * (i + 1), rr, g, :] = memsl[:, :, s]
    c["memstack"] = ms.reshape(128, S * 3 * 96)
    lmean, gmean = lm.mean(axis=1), gm.mean(axis=1)
    fms = np.zeros((96, 128))
    fms[:ML, :P] = lmean
    fms[ML:, P : 2 * P] = gmean
    fms[:ML, 64 : 64 + P] = 1.0
    fms[ML:, 64 + P : 128] = 1.0
    c["fms"] = fms
    # r-gate block first (cols 0:64) so v sits at base partition 0 next to h
    zrw = np.zeros((H + 1, 128))
    zrw[:H, :H] = Wr[1:]
    zrw[H, :H] = Wr[0]
    zrw[:H, H:] = Wz[1:]
    zrw[H, H:] = Wz[0]
    c["zrw"] = zrw
    Wzf = Wz[1:] + Wo @ Wz[0:1]
    Wrf = Wr[1:] + Wo @ Wr[0:1]
    c["zrwf"] = np.concatenate([Wrf, Wzf], axis=1)
    cws = np.zeros((H + 1, H))
    cws[:H] = 0.5 * Wc[1:]
    cws[H] = Wc[0]
    c["cws"] = cws
    c["cwf"] = 0.5 * Wc[1:]
    c["cwx"] = Wo @ Wc[0:1]
    c["qw"] = Wq.copy()
    owd = np.zeros((H, HORIZON * HORIZON))
    for dd in range(HORIZON):
        owd[:, HORIZON * dd + dd] = Wo[:, 0]
    c["owd"] = owd
    c["bqlog"] = np.concatenate([lm.sum(axis=1) @ bq, gm.sum(axis=1) @ bq]).reshape(96, 1)
    c["bzr2"] = (0.5 * np.concatenate([br, bz])).reshape(128, 1)
    c["bzrf2"] = (0.5 * np.concatenate([br + bo[0] * Wr[0], bz + bo[0] * Wz[0]])).reshape(128, 1)
    c["bce"] = bc.reshape(64, 1)
    c["bcd"] = (bc + bo[0] * Wc[0]).reshape(64, 1)
    c["bo"] = float(bo[0])
    return c


def _bf16(a):
    import ml_dtypes
    return np.ascontiguousarray(a).astype(ml_dtypes.bfloat16)


def _f32(a):
    return np.ascontiguousarray(a).astype(np.float32)


def make_in_maps(inp):
    c = precompute(inp)
    src = np.asarray(inp["source"], np.float32)
    shared = {
        "memstack": _bf16(c["memstack"]), "fms": _bf16(c["fms"]),
        "zrw": _bf16(c["zrw"]), "zrwf": _bf16(c["zrwf"]),
        "cws": _bf16(c["cws"]), "cwf": _bf16(c["cwf"]), "cwx": _bf16(c["cwx"]),
        "qw": _bf16(c["qw"]), "owd": _bf16(c["owd"]),
        "bqlog": _f32(c["bqlog"]), "bzr2": _f32(c["bzr2"]),
        "bzrf2": _f32(c["bzrf2"]), "bce": _f32(c["bce"]), "bcd": _f32(c["bcd"]),
    }
    in_maps = []
    for core in range(NCORES):
        nodes = slice(core * NL, (core + 1) * NL)
        xsc = _bf16(src[:, :, nodes, 0].transpose(1, 2, 0).reshape(T, NB))
        nswc = _bf16(c["nsw_full"][nodes].transpose(1, 0, 2).reshape(64, NL * 64))
        in_maps.append(dict(shared, xsrc=xsc, nsw=nswc))
    return in_maps


_BO_CACHE = {}


def assemble(results, bo=0.0):
    out = np.zeros((B, HORIZON, N, OUT), np.float32)
    for core in range(NCORES):
        nodes = slice(core * NL, (core + 1) * NL)
        ys = np.asarray(results[core]["ys"], np.float32) + bo  # [HORIZON, NB]
        out[:, :, nodes, 0] = ys.reshape(HORIZON, NL, B).transpose(2, 0, 1)
    return out


_NC_CACHE = {}


def kernel(**inputs):
    if "nc" not in _NC_CACHE:
        _NC_CACHE["nc"] = build_nc()
    nc = _NC_CACHE["nc"]
    in_maps = make_in_maps(inputs)
    bo = float(np.asarray(inputs["bo"], np.float64)[0])
    res = bass_utils.run_bass_kernel_spmd(nc, in_maps, core_ids=list(range(NCORES)))
    return assemble(res.results, bo)


# revision 26
# speedup vs baseline: 1.1345x; 1.1345x over previous
"""Trainium2 Bass kernel for nn_H_DYNA_42348377538865 (scatter_memory GRU + memory attention).

Self-contained: shards node dim N=512 across 8 NeuronCores (64 nodes/core),
runs a fully-unrolled 24-step recurrence per core, gathers on host.

Layout: feature-on-partitions, (node, batch) on free dim (col = n_local*32 + b,
NB=2048 cols/core). Key decompositions (validated vs reference in numpy):
  - rolling q-cache: q(h_t) computed once/step; 12 slots in 3x[128,2048] tiles;
    slot j pairs with memory slice s=(j-t)%12 -> 12 precomputed rotation stacks
  - softmax without max-subtraction; ONE fused matmul [96,128] emits means
    (rows 0:64) and replicated sums (rows 64:128) so the reciprocal broadcasts
  - single act table: sigmoid(x) = 0.5*(1+tanh(x/2)); exp/tanh/copy all live
    in the `exp_and_others` table -> zero table reloads
  - z/r gates in ONE [65,128] matmul + ONE tanh; GRU update via
    scalar_tensor_tensor: 2rh=(1+v)h (0.5 folded into Wc), w=hc-h, s=(1+u)w,
    h+=0.5s
  - q bias bq folded into the exp bias (sum_s bq.mem, constant per mem row)
  - decode: Wo folded into gate weights (no autoregressive y->x loopback);
    y accumulates in a persistent PSUM bank, copied out once at the end
  - hypernet nsw = node_emb @ weight_pool precomputed on host (param repack)
"""
import numpy as np
import sys

for _p in ("/opt/trn_rl_repo",):
    if _p not in sys.path:
        sys.path.append(_p)

import concourse.bass as bass
import concourse.bacc as bacc
import concourse.mybir as mybir
import concourse.tile as tile
from concourse import bass_utils

B, T, HORIZON, N = 32, 12, 12, 512
IN, OUT, H, P = 1, 1, 64, 32
S, ML, MG, DE = 12, 64, 32, 10
NCORES = 8
NL = N // NCORES        # 64
NB = NL * B             # 2048
NSTEP = T + HORIZON     # 24
CH = 4                  # column chunks
CW = NB // CH           # 512

F32 = mybir.dt.float32
BF16 = mybir.dt.bfloat16
AF = mybir.ActivationFunctionType
ALU = mybir.AluOpType


def build_nc():
    nc = bacc.Bacc("TRN2", target_bir_lowering=False, debug=False)
    d = {}
    d["xsrc"] = nc.dram_tensor("xsrc", [T, NB], BF16, kind="ExternalInput")
    d["memstack"] = nc.dram_tensor("memstack", [128, S * 3 * 96], BF16, kind="ExternalInput")
    d["nsw"] = nc.dram_tensor("nsw", [64, NL * 64], BF16, kind="ExternalInput")
    d["fms"] = nc.dram_tensor("fms", [96, 128], BF16, kind="ExternalInput")
    d["zrw"] = nc.dram_tensor("zrw", [65, 128], BF16, kind="ExternalInput")
    d["zrwf"] = nc.dram_tensor("zrwf", [64, 128], BF16, kind="ExternalInput")
    d["cws"] = nc.dram_tensor("cws", [65, 64], BF16, kind="ExternalInput")
    d["cwf"] = nc.dram_tensor("cwf", [64, 64], BF16, kind="ExternalInput")
    d["cwx"] = nc.dram_tensor("cwx", [64, 64], BF16, kind="ExternalInput")
    d["qw"] = nc.dram_tensor("qw", [64, 32], BF16, kind="ExternalInput")
    d["owd"] = nc.dram_tensor("owd", [64, HORIZON * HORIZON], BF16, kind="ExternalInput")
    d["bqlog"] = nc.dram_tensor("bqlog", [96, 1], F32, kind="ExternalInput")
    d["bzr2"] = nc.dram_tensor("bzr2", [128, 1], F32, kind="ExternalInput")
    d["bzrf2"] = nc.dram_tensor("bzrf2", [128, 1], F32, kind="ExternalInput")
    d["bce"] = nc.dram_tensor("bce", [64, 1], F32, kind="ExternalInput")
    d["bcd"] = nc.dram_tensor("bcd", [64, 1], F32, kind="ExternalInput")
    ys_d = nc.dram_tensor("ys", [HORIZON, NB], BF16, kind="ExternalOutput")

    with tile.TileContext(nc) as tc:
        with (
            tc.tile_pool(name="consts", bufs=1) as cp,
            tc.tile_pool(name="sp", bufs=4) as sp,
            tc.tile_pool(name="ps", bufs=2, space="PSUM") as pp,
            tc.tile_pool(name="pq", bufs=1, space="PSUM") as pq,
            tc.tile_pool(name="py", bufs=1, space="PSUM") as py,
        ):
            xs = cp.tile([T, NB], BF16)
            nc.sync.dma_start(xs[:], d["xsrc"].ap())
            msk = cp.tile([128, S * 3 * 96], BF16)
            nc.sync.dma_start(msk[:], d["memstack"].ap())
            nsw = cp.tile([64, NL * 64], BF16)
            nc.sync.dma_start(nsw[:], d["nsw"].ap())
            fms = cp.tile([96, 128], BF16)
            nc.sync.dma_start(fms[:], d["fms"].ap())
            zrw = cp.tile([65, 128], BF16)
            nc.sync.dma_start(zrw[:], d["zrw"].ap())
            zrwf = cp.tile([64, 128], BF16)
            nc.sync.dma_start(zrwf[:], d["zrwf"].ap())
            cws = cp.tile([65, 64], BF16)
            nc.sync.dma_start(cws[:], d["cws"].ap())
            cwf = cp.tile([64, 64], BF16)
            nc.sync.dma_start(cwf[:], d["cwf"].ap())
            cwx = cp.tile([64, 64], BF16)
            nc.sync.dma_start(cwx[:], d["cwx"].ap())
            qw = cp.tile([64, 32], BF16)
            nc.sync.dma_start(qw[:], d["qw"].ap())
            owd = cp.tile([64, HORIZON * HORIZON], BF16)
            nc.sync.dma_start(owd[:], d["owd"].ap())
            bqlog = cp.tile([96, 1], F32)
            nc.sync.dma_start(bqlog[:], d["bqlog"].ap())
            bzr2 = cp.tile([128, 1], F32)
            nc.sync.dma_start(bzr2[:], d["bzr2"].ap())
            bzrf2 = cp.tile([128, 1], F32)
            nc.sync.dma_start(bzrf2[:], d["bzrf2"].ap())
            bce = cp.tile([64, 1], F32)
            nc.sync.dma_start(bce[:], d["bce"].ap())
            bcd = cp.tile([64, 1], F32)
            nc.sync.dma_start(bcd[:], d["bcd"].ap())

            qb = []
            for g in range(3):
                q = cp.tile([128, NB], BF16, name=f"qb{g}")
                nc.vector.memset(q[:], 0.0)
                qb.append(q)
            hx = cp.tile([65, NB], BF16)
            nc.vector.memset(hx[:], 0.0)
            rhx = cp.tile([65, NB], BF16)
            nc.vector.memset(rhx[:], 0.0)
            # y staging: row 32c + d holds decode step d of chunk c
            # (32-aligned chunk bases for the ACT copy)
            ysb = cp.tile([128, CW], BF16)
            nc.sync.dma_start(hx[64:65, :], xs[0:1, :])
            nc.sync.dma_start(rhx[64:65, :], xs[0:1, :])

            # persistent PSUM: q projections (4 chunks x 32 rows), and the
            # decode y accumulator (chunk c rows 32c:32c+12; each decode step
            # adds Wo^T h into row 32c+d and +0 elsewhere)
            qpb = pq.tile([128, CW], F32)
            ypt = py.tile([128, CW], F32)

            csl = [slice(c * CW, (c + 1) * CW) for c in range(CH)]

            for t in range(NSTEP):
                r = t % S
                j = t % S
                g_w, row_w = j // 4, (j % 4) * 32
                enc = t <= T  # t==12 still uses x-row (x = source[:, -1])

                # --- PE: z|r gate logits ---
                zrp = []
                for c in range(CH):
                    zp = pp.tile([128, CW], F32, tag="fz")
                    if enc:
                        nc.tensor.matmul(zp[:], zrw[:], hx[:, csl[c]],
                                         start=True, stop=True)
                    else:
                        nc.tensor.matmul(zp[:], zrwf[:], hx[0:64, csl[c]],
                                         start=True, stop=True)
                    zrp.append(zp)
                # --- PE: attention logits from q-cache ---
                lgp = []
                for c in range(CH):
                    lg = pp.tile([96, CW], F32, tag="lg")
                    for g in range(3):
                        off = (r * 3 + g) * 96
                        nc.tensor.matmul(
                            lg[:], msk[:, off : off + 96], qb[g][:, csl[c]],
                            start=(g == 0), stop=(g == 2),
                        )
                    lgp.append(lg)
                # --- ACT: gates u|v = tanh((logits+b)/2) ---
                uvl = []
                for c in range(CH):
                    uv = sp.tile([128, CW], BF16, tag="uv", bufs=6)
                    nc.scalar.activation(uv[:], zrp[c][:], AF.Tanh,
                                         bias=(bzr2 if enc else bzrf2)[:, 0:1],
                                         scale=0.5)
                    uvl.append(uv)
                # --- ACT: exp of attention logits ---
                exl = []
                for c in range(CH):
                    ex = sp.tile([96, CW], BF16, tag="ex", bufs=6)
                    nc.scalar.activation(ex[:], lgp[c][:], AF.Exp,
                                         bias=bqlog[:, 0:1])
                    exl.append(ex)
                # --- DVE: 2*r*h = (1+v)*h via ts(4x) + tt(2x) ---
                vpl = []
                for c in range(CH):
                    vp = sp.tile([64, CW], BF16, tag="vp", bufs=5)
                    nc.vector.tensor_scalar(vp[:], uvl[c][0:64, :], 1.0, None, ALU.add)
                    vpl.append(vp)
                for c in range(CH):
                    nc.vector.tensor_mul(rhx[0:64, csl[c]], vpl[c][:],
                                         hx[0:64, csl[c]])
                # --- PE: fused mean/sum matmul ---
                fzl = []
                for c in range(CH):
                    fz = pp.tile([128, CW], F32, tag="fz")
                    nc.tensor.matmul(fz[:], fms[:], exl[c][:],
                                     start=True, stop=True)
                    fzl.append(fz)
                # --- DVE: reciprocal of sums; fn = means * recip ---
                rtl = []
                for c in range(CH):
                    rt = sp.tile([64, CW], F32, tag="rt", bufs=5)
                    nc.vector.reciprocal_approx_fast(rt[:], fzl[c][64:128, :])
                    rtl.append(rt)
                fnl = []
                for c in range(CH):
                    fn = sp.tile([64, CW], BF16, tag="fn", bufs=5)
                    nc.vector.tensor_mul(fn[:], fzl[c][0:64, :], rtl[c][:])
                    fnl.append(fn)
                # --- PE: candidate pre-activation: Wc part then hypernet ---
                accl = []
                for c in range(CH):
                    acc = pp.tile([64, CW], F32, tag="acc")
                    if enc:
                        nc.tensor.matmul(acc[:], cws[:], rhx[:, csl[c]],
                                         start=True, stop=False,
                                         skip_group_check=True)
                    else:
                        nc.tensor.matmul(acc[:], cwf[:], rhx[0:64, csl[c]],
                                         start=True, stop=False,
                                         skip_group_check=True)
                        nc.tensor.matmul(acc[:], cwx[:], hx[0:64, csl[c]],
                                         start=False, stop=False,
                                         skip_group_check=True)
                    accl.append(acc)
                for c in range(CH):
                    for k in range(16):
                        n = c * 16 + k
                        nc.tensor.matmul(
                            accl[c][:, k * 32 : (k + 1) * 32],
                            nsw[:, n * 64 : (n + 1) * 64],
                            fnl[c][:, k * 32 : (k + 1) * 32],
                            start=False, stop=(k == 15), skip_group_check=True,
                        )
                # --- ACT: hc = tanh(acc + bc) ---
                hcl = []
                for c in range(CH):
                    hc = sp.tile([64, CW], BF16, tag="hc", bufs=5)
                    nc.scalar.activation(hc[:], accl[c][:], AF.Tanh,
                                         bias=(bce if enc else bcd)[:, 0:1])
                    hcl.append(hc)
                # --- h += (0.5+0.5u)*(hc-h): ts folds the 0.5s, tt does the rest;
                # up's out moves u from base 64 to base 0 so all tt inputs align ---
                upl = []
                for c in range(CH):
                    up = sp.tile([64, CW], BF16, tag="up", bufs=5)
                    nc.vector.tensor_scalar(up[:], uvl[c][64:128, :], 0.5, 0.5,
                                            ALU.mult, ALU.add)
                    upl.append(up)
                wl = []
                for c in range(CH):
                    w = sp.tile([64, CW], BF16, tag="w", bufs=5)
                    nc.gpsimd.tensor_sub(w[:], hcl[c][:], hx[0:64, csl[c]])
                    wl.append(w)
                sl = []
                for c in range(CH):
                    s2 = sp.tile([64, CW], BF16, tag="s2", bufs=5)
                    nc.vector.tensor_mul(s2[:], upl[c][:], wl[c][:])
                    sl.append(s2)
                for c in range(CH):
                    nc.vector.tensor_add(hx[0:64, csl[c]], hx[0:64, csl[c]],
                                         sl[c][:])
                # --- PE: q projection of new h; decode y projection ---
                if t < NSTEP - 1:
                    for c in range(CH):
                        nc.tensor.matmul(
                            qpb[32 * c : 32 * (c + 1), :], qw[:],
                            hx[0:64, csl[c]], start=True, stop=True,
                            tile_position=(0, 32 * c),
                        )
                if t >= T:
                    dstep = t - T
                    for c in range(CH):
                        nc.tensor.matmul(
                            ypt[32 * c : 32 * c + HORIZON, :],
                            owd[:, HORIZON * dstep : HORIZON * (dstep + 1)],
                            hx[0:64, csl[c]],
                            start=(dstep == 0), stop=(dstep == HORIZON - 1),
                            skip_group_check=True,
                            tile_position=(0, 32 * c),
                        )
                # --- DVE/ACT: q-cache slot update (gpsimd cannot read PSUM) ---
                if t < NSTEP - 1:
                    for c in range(CH):
                        dst = qb[g_w][row_w : row_w + 32, csl[c]]
                        src = qpb[32 * c : 32 * (c + 1), :]
                        if c < 2:
                            nc.vector.tensor_copy(dst, src)
                        else:
                            nc.scalar.activation(dst, src, AF.Copy)
                # --- DMA: encode x prefetch ---
                if t < T - 1:
                    nc.sync.dma_start(hx[64:65, :], xs[t + 1 : t + 2, :])
                    nc.sync.dma_start(rhx[64:65, :], xs[t + 1 : t + 2, :])

            for c in range(CH):
                nc.scalar.activation(
                    ysb[32 * c : 32 * c + HORIZON, :],
                    ypt[32 * c : 32 * c + HORIZON, :], AF.Copy)
            for c in range(CH):
                nc.sync.dma_start(
                    ys_d.ap()[0:HORIZON, c * CW : (c + 1) * CW],
                    ysb[32 * c : 32 * c + HORIZON, :])
    nc.compile()
    return nc


def precompute(inp):
    lm = np.asarray(inp["local_mem"], np.float64)
    gm = np.asarray(inp["global_mem"], np.float64)
    Wq = np.asarray(inp["Wq"], np.float64)
    bq = np.asarray(inp["bq"], np.float64)
    node_emb = np.asarray(inp["node_emb"], np.float64)
    wp = np.asarray(inp["weight_pool"], np.float64)
    Wz = np.asarray(inp["Wz"], np.float64)
    bz = np.asarray(inp["bz"], np.float64)
    Wr = np.asarray(inp["Wr"], np.float64)
    br = np.asarray(inp["br"], np.float64)
    Wc = np.asarray(inp["Wc"], np.float64)
    bc = np.asarray(inp["bc"], np.float64)
    Wo = np.asarray(inp["Wo"], np.float64)
    bo = np.asarray(inp["bo"], np.float64)

    c = {}
    c["nsw_full"] = np.einsum("nd,dfh->nfh", node_emb, wp)
    memsl = np.concatenate([lm.transpose(2, 0, 1), gm.transpose(2, 0, 1)], axis=1)  # [P,96,S]
    ms = np.zeros((128, S, 3, 96))
    for rr in range(S):
        for g in range(3):
            for i in range(4):
                s = (4 * g + i - rr) % S
                ms[32 * i : 32 * (i + 1), rr, g, :] = memsl[:, :, s]
    c["memstack"] = ms.reshape(128, S * 3 * 96)
    lmean, gmean = lm.mean(axis=1), gm.mean(axis=1)
    fms = np.zeros((96, 128))
    fms[:ML, :P] = lmean
    fms[ML:, P : 2 * P] = gmean
    fms[:ML, 64 : 64 + P] = 1.0
    fms[ML:, 64 + P : 128] = 1.0
    c["fms"] = fms
    # r-gate block first (cols 0:64) so v sits at base partition 0 next to h
    zrw = np.zeros((H + 1, 128))
    zrw[:H, :H] = Wr[1:]
    zrw[H, :H] = Wr[0]
    zrw[:H, H:] = Wz[1:]
    zrw[H, H:] = Wz[0]
    c["zrw"] = zrw
    Wzf = Wz[1:] + Wo @ Wz[0:1]
    Wrf = Wr[1:] + Wo @ Wr[0:1]
    c["zrwf"] = np.concatenate([Wrf, Wzf], axis=1)
    cws = np.zeros((H + 1, H))
    cws[:H] = 0.5 * Wc[1:]
    cws[H] = Wc[0]
    c["cws"] = cws
    c["cwf"] = 0.5 * Wc[1:]
    c["cwx"] = Wo @ Wc[0:1]
    c["qw"] = Wq.copy()
    owd = np.zeros((H, HORIZON * HORIZON))
    for dd in range(HORIZON):
        owd[:, HORIZON * dd + dd] = Wo[:, 0]
    c["owd"] = owd
    c["bqlog"] = np.concatenate([lm.sum(axis=1) @ bq, gm.sum(axis=1) @ bq]).reshape(96, 1)
    c["bzr2"] = (0.5 * np.concatenate([br, bz])).reshape(128, 1)
    c["bzrf2"] = (0.5 * np.concatenate([br + bo[0] * Wr[0], bz + bo[0] * Wz[0]])).reshape(128, 1)
    c["bce"] = bc.reshape(64, 1)
    c["bcd"] = (bc + bo[0] * Wc[0]).reshape(64, 1)
    c["bo"] = float(bo[0])
    return c


def _bf16(a):
    import ml_dtypes
    return np.ascontiguousarray(a).astype(ml_dtypes.bfloat16)


def _f32(a):
    return np.ascontiguousarray(a).astype(np.float32)


def make_in_maps(inp):
    c = precompute(inp)
    src = np.asarray(inp["source"], np.float32)
    shared = {
        "memstack": _bf16(c["memstack"]), "fms": _bf16(c["fms"]),
        "zrw": _bf16(c["zrw"]), "zrwf": _bf16(c["zrwf"]),
        "cws": _bf16(c["cws"]), "cwf": _bf16(c["cwf"]), "cwx": _bf16(c["cwx"]),
        "qw": _bf16(c["qw"]), "owd": _bf16(c["owd"]),
        "bqlog": _f32(c["bqlog"]), "bzr2": _f32(c["bzr2"]),
        "bzrf2": _f32(c["bzrf2"]), "bce": _f32(c["bce"]), "bcd": _f32(c["bcd"]),
    }
    in_maps = []
    for core in range(NCORES):
        nodes = slice(core * NL, (core + 1) * NL)
        xsc = _bf16(src[:, :, nodes, 0].transpose(1, 2, 0).reshape(T, NB))
        nswc = _bf16(c["nsw_full"][nodes].transpose(1, 0, 2).reshape(64, NL * 64))
        in_maps.append(dict(shared, xsrc=xsc, nsw=nswc))
    return in_maps


_BO_CACHE = {}


def assemble(results, bo=0.0):
    out = np.zeros((B, HORIZON, N, OUT), np.float32)
    for core in range(NCORES):
        nodes = slice(core * NL, (core + 1) * NL)
        ys = np.asarray(results[core]["ys"], np.float32) + bo  # [HORIZON, NB]
        out[:, :, nodes, 0] = ys.reshape(HORIZON, NL, B).transpose(2, 0, 1)
    return out


_NC_CACHE = {}


def kernel(**inputs):
    if "nc" not in _NC_CACHE:
        _NC_CACHE["nc"] = build_nc()
    nc = _NC_CACHE["nc"]
    in_maps = make_in_maps(inputs)
    bo = float(np.asarray(inputs["bo"], np.float64)[0])
    res = bass_utils.run_bass_kernel_spmd(nc, in_maps, core_ids=list(range(NCORES)))
    return assemble(res.results, bo)
